# revision 29
# baseline (speedup 1.0000x reference)
"""Block-diagonal rotation (COB) kernel for Trainium2, 8 NeuronCores.

Computes out[..., block_i] = x[..., block_i] @ W_i.T for 8 square blocks of
sizes [512, 1024, 256, 768, 384, 640, 128, 384] (features sum to 4096),
x shape (4, 2048, 4096) fp32.

Strategy (bf16 end-to-end, data-parallel over rows):
  - 8192 rows split 8 ways (1024 rows/core); each core holds all weights.
  - Host converts x and the pre-transposed weights to bf16 and upcasts the
    bf16 output back to fp32 (harness tolerance is 2e-2; bf16 end-to-end
    lands ~3.9e-3).  This halves HBM traffic vs fp32: 21.1 MiB/core
    (x-in 8 + w 5.1 + out 8) and makes bf16 PE transposes 1 cycle/row.
  - x tiles [128, 4096] are DMA'd naturally (rows on partitions),
    transposed 128x128 on the TensorEngine against a DMA'd identity,
    PSUM->SBUF copied by the DVE, then used as the stationary operand of
    bf16 matmuls against SBUF-resident weight tiles (PSUM fp32 accum).
  - PSUM results are downcast-copied to bf16 staging tiles (alternating
    DVE/ACT) and stored as 0.5 MiB DMAs; the final row-tile streams out
    per-slice so the tail overlaps compute.
  - Transposes are emitted ONE at a time between matmuls (budget pump,
    ~240 matmul-cycles of cover per transpose) with a 2-row-tile
    lookahead window, so their stationary loads hide under matmul
    streaming.
  - Prologue is DMA-ramp limited (~120 GB/s for the first ~25 us), so:
    weights split across both HWDGE rings (even k-chunks on scalar, odd
    on sync interleaved with the x prefetches, in consumption order) and
    the 2 MiB w1 block is processed LAST in each row-tile (B_ORDER),
    giving its preload ~7 us of extra slack inside the ramp window.

Measured on trn2 (8 cores): ~116-120 us HW exec cold (device shows
+-10-15% thermal/neighbor variance; identical builds measured 115.3-134),
max rel err ~3.9e-3.  PE busy ~92-95 us vs an 83.6 us streaming floor
(167,936 matmul rows + 32,768 transpose rows @ 2.4 GHz); remaining loss
is the fixed ~9 us framework init, DMA-ramp-limited prologue stalls
(~5 us), and the ~6 us teardown drain.
"""

import numpy as np
import ml_dtypes

import concourse.bacc as bacc
import concourse.mybir as mybir
from concourse.tile import TileContext
from concourse.bass_utils import run_bass_kernel_spmd

SIZES = [512, 1024, 256, 768, 384, 640, 128, 384]
OFFS = np.cumsum([0] + SIZES)
N_CORES = 8
ROWS_TOTAL = 4 * 2048
ROWS_PER_CORE = ROWS_TOTAL // N_CORES  # 1024
D = 4096
P = 128
R_TILES = ROWS_PER_CORE // P  # 8

# e-slices per block: PSUM bank holds 512 fp32 per partition
E_SLICES = {
    512: [512], 1024: [512, 512], 256: [256], 768: [512, 256],
    384: [384], 640: [384, 256], 128: [128],
}

BF16 = mybir.dt.bfloat16
F32 = mybir.dt.float32

_cache = {}


def build_nc():
    if "nc" in _cache:
        return _cache["nc"]
    nc = bacc.Bacc()
    x_d = nc.declare_dram_parameter("x", [ROWS_PER_CORE, D], BF16, isOutput=False)
    w_d = [
        nc.declare_dram_parameter(f"w{i}", [s, s], BF16, isOutput=False)
        for i, s in enumerate(SIZES)
    ]
    id_d = nc.declare_dram_parameter("ident", [P, P], BF16, isOutput=False)
    out_d = nc.declare_dram_parameter("out", [ROWS_PER_CORE, D], BF16, isOutput=True)

    x_v = x_d.rearrange("(r p) d -> r p d", p=P)
    out_v = out_d.rearrange("(r p) d -> r p d", p=P)

    with TileContext(nc) as tc:
        with (
            tc.tile_pool(name="wres", bufs=1) as wres,
            tc.tile_pool(name="xnat", bufs=2) as xnat_p,
            tc.tile_pool(name="xt", bufs=3) as xt_p,
            tc.tile_pool(name="osb", bufs=2) as osb_p,
            tc.tile_pool(name="xb", bufs=3) as xb_p,
            tc.tile_pool(name="idp", bufs=1) as idp,
            tc.tile_pool(name="tp", bufs=2, space="PSUM") as tp_p,
            tc.tile_pool(name="mm", bufs=4, space="PSUM") as mm_p,
        ):
            # identity (bf16) for PE transpose — DMA'd from DRAM so the
            # first transpose doesn't wait on DVE table loads / iota setup
            ident = idp.tile([P, P], BF16, tag="idb")
            nc.scalar.dma_start(out=ident[:], in_=id_d[:, :])

            # resident weights: per block, per k-tile: [128, s] bf16.
            # Even k-chunks stream on the scalar ring immediately (it is
            # otherwise idle in the prologue); odd k-chunks go on the sync
            # ring interleaved between the x-tile loads (deferred below) so
            # each half-weight stream finishes just ahead of first use.
            wt = [None] * len(SIZES)
            w_sync_dmas = {i: [] for i in range(len(SIZES))}
            ci = 0
            for i in [0, 2, 3, 4, 5, 6, 7, 1]:
                s = SIZES[i]
                w_v = w_d[i].rearrange("(k p) e -> k p e", p=P)
                ks = []
                for k in range(s // P):
                    t = wres.tile([P, s], BF16, tag=f"w{i}_{k}")
                    if ci % 2 == 0:
                        nc.scalar.dma_start(out=t[:], in_=w_v[k])
                    else:
                        w_sync_dmas[i].append((t, w_v[k]))
                    ks.append(t)
                    ci += 1
                wt[i] = ks

            # Software pipeline over row-tiles (demand-driven transpose pump,
            # sliding window of up to 2 row-tiles of transposed x).
            xnat = {}
            xts_all = {}

            def issue_x_dma(r, chunks=2):
                # row-tiles >= 2 only need cols 0-2047 naturally: chunks
                # 16-31 arrive pre-transposed via XBAR DMAs (issue_xbar_pair)
                xn = xnat_p.tile([P, D], BF16, tag="xn", name="xnt")
                width = D if r < 2 else D // 2
                q = width // chunks
                for c in range(chunks):
                    nc.sync.dma_start(out=xn[:, c * q:(c + 1) * q],
                                      in_=x_v[r][:, c * q:(c + 1) * q])
                xnat[r] = xn

            # XBAR transpose loads: for a row-PAIR p (row-tiles 2p, 2p+1),
            # load x^T chunks 16-31 straight from DRAM on the scalar ring
            # ([256,128] -> [128,256], ~1.2us of ACT sequencer each).  The
            # scalar ring is idle after the weight preload, and each PE
            # transpose removed saves ~65 ns of TensorEngine stream time.
            xbar_xts = {}  # pair -> {chunk: tile}

            def issue_xbar_pair(p):
                m = {}
                for c in range(16, 32):
                    t = xb_p.tile([P, 2 * P], BF16, tag=f"xb{c}", name="xbt")
                    nc.scalar.dma_start(
                        out=t[:],
                        in_=x_d[2 * P * p:2 * P * (p + 1), c * P:(c + 1) * P],
                        transpose=True,
                    )
                    m[c] = t
                xbar_xts[p] = m

            # Transposes are LDWEIGHTS-bound on the PE (the 128-row
            # stationary load takes ~2x the 128-row identity stream), so we
            # emit them ONE at a time interleaved between block matmuls:
            # each matmul with nw>=256 fully hides one transpose's LD.
            # State: per (r, j) group, a psum tile filled by 4 single
            # transposes; after the 4th, a DVE copy publishes the xt tile.
            tp_state = {}  # (r, j) -> [psum_tile, count]

            def emit_one_transpose(r, j, i):
                src = xnat[r]
                base = P * 4 * j
                if (r, j) not in tp_state:
                    tp_state[(r, j)] = [tp_p.tile([P, 4 * P], BF16, tag="tp", name="tpps"), 0]
                ps, _ = tp_state[(r, j)]
                nc.tensor.transpose(
                    ps[:, P * i:P * (i + 1)],
                    src[:, base + P * i:base + P * (i + 1)],
                    ident[:],
                )
                tp_state[(r, j)][1] += 1
                if tp_state[(r, j)][1] == 4:
                    xt = xt_p.tile([P, 4 * P], BF16, tag=f"xt{j}", name="xtt")
                    nc.vector.tensor_copy(xt[:], ps[:])
                    xts_all.setdefault(r, {})[j] = xt
                    del tp_state[(r, j)]

            tp_queue = [(r, j, i)
                        for r in range(R_TILES)
                        for j in range(8 if r < 2 else 4)
                        for i in range(4)]
            # pos_end[(r, j)] = queue index once group (r, j) is complete
            pos_end = {}
            for idx, (r_, j_, i_) in enumerate(tp_queue):
                pos_end[(r_, j_)] = idx + 1
            row_end_pos = {r_: 0 for r_ in range(R_TILES)}
            for (r_, j_), e in pos_end.items():
                row_end_pos[r_] = max(row_end_pos[r_], e)
            state = {"cursor": 0}

            def pump_to(idx):
                # emit single transposes up to global index idx (exclusive)
                idx = min(idx, len(tp_queue))
                while state["cursor"] < idx:
                    r_, j_, i_ = tp_queue[state["cursor"]]
                    emit_one_transpose(r_, j_, i_)
                    state["cursor"] += 1

            # j-group needed to cover all d-tiles of block b
            J_HI = [(int(OFFS[b + 1]) - 1) // 512 for b in range(len(SIZES))]

            # process the 2 MiB w1 block LAST in each row-tile: its preload
            # gets ~7 extra us inside the DMA ramp-up window
            B_ORDER = [0, 2, 3, 4, 5, 6, 7, 1]


            # Sync-ring prologue order, matched to PE consumption:
            # x0 first (transposes start ASAP), then the odd chunks of the
            # first two blocks' weights (needed by the earliest matmuls),
            # then x1, then the remaining odd weight chunks, interleaved
            # ahead of the steady-state x prefetches.
            issue_x_dma(0, chunks=4)
            for t, src in w_sync_dmas[0]:
                nc.sync.dma_start(out=t[:], in_=src)
            issue_x_dma(1)
            for i in (2, 3, 4, 5, 6, 7, 1):
                for t, src in w_sync_dmas[i]:
                    nc.sync.dma_start(out=t[:], in_=src)

            issue_xbar_pair(1)

            # budget-driven interleave: one transpose LD (~99 ns) hides
            # under ~240 matmul-stream cycles
            T_COVER_CYCLES = 240
            budget = {"c": 0}

            def interleave_pump(nw, cap):
                budget["c"] += nw
                while (budget["c"] >= T_COVER_CYCLES
                       and state["cursor"] < min(cap, len(tp_queue))):
                    r_, j_, i_ = tp_queue[state["cursor"]]
                    emit_one_transpose(r_, j_, i_)
                    state["cursor"] += 1
                    budget["c"] -= T_COVER_CYCLES

            for r in range(R_TILES):
                last = r == R_TILES - 1
                if r + 2 < R_TILES:
                    issue_x_dma(r + 2)
                if r == 0:
                    issue_xbar_pair(2)
                elif r == 2:
                    issue_xbar_pair(3)
                cap = row_end_pos[min(r + 2, R_TILES - 1)]
                o_t = osb_p.tile([P, D], BF16, tag="os")
                for b in B_ORDER:
                    s = SIZES[b]
                    jn = J_HI[b] if r < 2 else min(J_HI[b], 3)
                    pump_to(pos_end[(r, jn)])
                    xts = xts_all[r]
                    d0 = int(OFFS[b]) // P
                    kt = s // P
                    n0 = 0
                    for nw in E_SLICES[s]:
                        ps = mm_p.tile([P, nw], F32, tag="mm", name="mmps")
                        for k in range(kt):
                            g = d0 + k
                            if r < 2 or g < 16:
                                lhsT = xts[g // 4][:, P * (g % 4):P * (g % 4 + 1)]
                            else:
                                lhsT = xbar_xts[r // 2][g][
                                    :, (r % 2) * P:(r % 2) * P + P]
                            nc.tensor.matmul(
                                ps[:], lhsT, wt[b][k][:, n0:n0 + nw],
                                start=(k == 0), stop=(k == kt - 1),
                            )
                            interleave_pump(nw, cap)
                        dst = o_t[:, int(OFFS[b]) + n0:int(OFFS[b]) + n0 + nw]
                        nc.vector.tensor_copy(dst, ps[:])
                        if last:
                            # stream the final row-tile out per-slice so the
                            # tail store overlaps the remaining compute
                            c0 = int(OFFS[b]) + n0
                            nc.sync.dma_start(out=out_v[r][:, c0:c0 + nw],
                                               in_=o_t[:, c0:c0 + nw])
                        n0 += nw
                del xts_all[r]
                if not last:
                    nc.sync.dma_start(out=out_v[r][:, :D // 2], in_=o_t[:, :D // 2])
                    nc.sync.dma_start(out=out_v[r][:, D // 2:], in_=o_t[:, D // 2:])

    nc.finalize()
    _cache["nc"] = nc
    return nc


def build_in_maps(x, w0, w1, w2, w3, w4, w5, w6, w7):
    x = np.asarray(x, dtype=np.float32).reshape(ROWS_TOTAL, D)
    xb = x.astype(ml_dtypes.bfloat16)
    ws = [w0, w1, w2, w3, w4, w5, w6, w7]
    wts = [
        np.ascontiguousarray(np.asarray(w, dtype=np.float32).T).astype(
            ml_dtypes.bfloat16
        )
        for w in ws
    ]
    ident = np.eye(P, dtype=np.float32).astype(ml_dtypes.bfloat16)
    in_maps = []
    for c in range(N_CORES):
        m = {"x": xb[c * ROWS_PER_CORE:(c + 1) * ROWS_PER_CORE], "ident": ident}
        for i, wtb in enumerate(wts):
            m[f"w{i}"] = wtb
        in_maps.append(m)
    return in_maps


def kernel(x, w0, w1, w2, w3, w4, w5, w6, w7):
    nc = build_nc()
    in_maps = build_in_maps(x, w0, w1, w2, w3, w4, w5, w6, w7)
    res = run_bass_kernel_spmd(nc, in_maps, list(range(N_CORES)))
    out = np.concatenate([r["out"] for r in res.results], axis=0)
    return out.reshape(4, 2048, D).astype(np.float32)



# revision 30
# speedup vs baseline: 1.1913x; 1.1913x over previous
"""Block-diagonal rotation (COB) kernel for Trainium2, 8 NeuronCores.

Computes out[..., block_i] = x[..., block_i] @ W_i.T for 8 square blocks of
sizes [512, 1024, 256, 768, 384, 640, 128, 384] (features sum to 4096),
x shape (4, 2048, 4096) fp32.

Strategy (bf16 end-to-end, data-parallel over rows):
  - 8192 rows split 8 ways (1024 rows/core); each core holds all weights.
  - Host converts x and the pre-transposed weights to bf16 and upcasts the
    bf16 output back to fp32 (harness tolerance is 2e-2; bf16 end-to-end
    lands ~3.9e-3).  This halves HBM traffic vs fp32: 21.1 MiB/core
    (x-in 8 + w 5.1 + out 8) and makes bf16 PE transposes 1 cycle/row.
  - x tiles [128, 4096] are DMA'd naturally (rows on partitions),
    transposed 128x128 on the TensorEngine against a DMA'd identity,
    PSUM->SBUF copied by the DVE, then used as the stationary operand of
    bf16 matmuls against SBUF-resident weight tiles (PSUM fp32 accum).
  - PSUM results are downcast-copied to bf16 staging tiles (alternating
    DVE/ACT) and stored as 0.5 MiB DMAs; the final row-tile streams out
    per-slice so the tail overlaps compute.
  - Transposes are emitted ONE at a time between matmuls (budget pump,
    ~240 matmul-cycles of cover per transpose) with a 2-row-tile
    lookahead window, so their stationary loads hide under matmul
    streaming.
  - Prologue is DMA-ramp limited (~120 GB/s for the first ~25 us), so:
    weights split across both HWDGE rings (even k-chunks on scalar, odd
    on sync interleaved with the x prefetches, in consumption order) and
    the 2 MiB w1 block is processed LAST in each row-tile (B_ORDER),
    giving its preload ~7 us of extra slack inside the ramp window.

Measured on trn2 (8 cores): ~116-120 us HW exec cold (device shows
+-10-15% thermal/neighbor variance; identical builds measured 115.3-134),
max rel err ~3.9e-3.  PE busy ~92-95 us vs an 83.6 us streaming floor
(167,936 matmul rows + 32,768 transpose rows @ 2.4 GHz); remaining loss
is the fixed ~9 us framework init, DMA-ramp-limited prologue stalls
(~5 us), and the ~6 us teardown drain.
"""

import numpy as np
import ml_dtypes

import concourse.bacc as bacc
import concourse.mybir as mybir
from concourse.tile import TileContext
from concourse.bass_utils import run_bass_kernel_spmd

SIZES = [512, 1024, 256, 768, 384, 640, 128, 384]
OFFS = np.cumsum([0] + SIZES)
N_CORES = 8
ROWS_TOTAL = 4 * 2048
ROWS_PER_CORE = ROWS_TOTAL // N_CORES  # 1024
D = 4096
P = 128
R_TILES = ROWS_PER_CORE // P  # 8

# e-slices per block: PSUM bank holds 512 fp32 per partition
E_SLICES = {
    512: [512], 1024: [512, 512], 256: [256], 768: [512, 256],
    384: [384], 640: [384, 256], 128: [128],
}

BF16 = mybir.dt.bfloat16
F32 = mybir.dt.float32

_cache = {}


def build_nc():
    if "nc" in _cache:
        return _cache["nc"]
    nc = bacc.Bacc()
    x_d = nc.declare_dram_parameter("x", [ROWS_PER_CORE, D], BF16, isOutput=False)
    w_d = [
        nc.declare_dram_parameter(f"w{i}", [s, s], BF16, isOutput=False)
        for i, s in enumerate(SIZES)
    ]
    id_d = nc.declare_dram_parameter("ident", [P, P], BF16, isOutput=False)
    out_d = nc.declare_dram_parameter("out", [ROWS_PER_CORE, D], BF16, isOutput=True)

    x_v = x_d.rearrange("(r p) d -> r p d", p=P)
    out_v = out_d.rearrange("(r p) d -> r p d", p=P)

    with TileContext(nc) as tc:
        with (
            tc.tile_pool(name="wres", bufs=1) as wres,
            tc.tile_pool(name="xnat", bufs=2) as xnat_p,
            tc.tile_pool(name="xt", bufs=3) as xt_p,
            tc.tile_pool(name="osb", bufs=2) as osb_p,
            tc.tile_pool(name="xb", bufs=2) as xb_p,
            tc.tile_pool(name="idp", bufs=1) as idp,
            tc.tile_pool(name="tp", bufs=2, space="PSUM") as tp_p,
            tc.tile_pool(name="mm", bufs=4, space="PSUM") as mm_p,
        ):
            # identity (bf16) for PE transpose — DMA'd from DRAM so the
            # first transpose doesn't wait on DVE table loads / iota setup
            ident = idp.tile([P, P], BF16, tag="idb")
            nc.scalar.dma_start(out=ident[:], in_=id_d[:, :])

            # resident weights: per block, per k-tile: [128, s] bf16.
            # Even k-chunks stream on the scalar ring immediately (it is
            # otherwise idle in the prologue); odd k-chunks go on the sync
            # ring interleaved between the x-tile loads (deferred below) so
            # each half-weight stream finishes just ahead of first use.
            wt = [None] * len(SIZES)
            w_sync_dmas = {i: [] for i in range(len(SIZES))}
            ci = 0
            for i in [0, 2, 3, 4, 5, 6, 7, 1]:
                s = SIZES[i]
                w_v = w_d[i].rearrange("(k p) e -> k p e", p=P)
                ks = []
                for k in range(s // P):
                    t = wres.tile([P, s], BF16, tag=f"w{i}_{k}")
                    if ci % 2 == 0:
                        nc.scalar.dma_start(out=t[:], in_=w_v[k])
                    else:
                        w_sync_dmas[i].append((t, w_v[k]))
                    ks.append(t)
                    ci += 1
                wt[i] = ks

            # Software pipeline over row-tiles (demand-driven transpose pump,
            # sliding window of up to 2 row-tiles of transposed x).
            xnat = {}
            xts_all = {}

            def issue_x_dma(r, chunks=2):
                # row-tiles >= 2 only need cols 0-2047 naturally: chunks
                # 16-31 arrive pre-transposed via XBAR DMAs (issue_xbar_pair)
                xn = xnat_p.tile([P, D], BF16, tag="xn", name="xnt")
                width = D if r < 4 else D // 2
                q = width // chunks
                for c in range(chunks):
                    nc.sync.dma_start(out=xn[:, c * q:(c + 1) * q],
                                      in_=x_v[r][:, c * q:(c + 1) * q])
                xnat[r] = xn

            # XBAR transpose loads: for a row-PAIR p (row-tiles 2p, 2p+1),
            # load x^T chunks 16-31 straight from DRAM on the scalar ring
            # ([256,128] -> [128,256], ~1.2us of ACT sequencer each).  The
            # scalar ring is idle after the weight preload, and each PE
            # transpose removed saves ~65 ns of TensorEngine stream time.
            xbar_xts = {}  # pair -> {chunk: tile}

            def issue_xbar_pair(p):
                m = {}
                for c in range(16, 32):
                    t = xb_p.tile([P, 2 * P], BF16, tag=f"xb{c}", name="xbt")
                    nc.scalar.dma_start(
                        out=t[:],
                        in_=x_d[2 * P * p:2 * P * (p + 1), c * P:(c + 1) * P],
                        transpose=True,
                    )
                    m[c] = t
                xbar_xts[p] = m

            # Transposes are LDWEIGHTS-bound on the PE (the 128-row
            # stationary load takes ~2x the 128-row identity stream), so we
            # emit them ONE at a time interleaved between block matmuls:
            # each matmul with nw>=256 fully hides one transpose's LD.
            # State: per (r, j) group, a psum tile filled by 4 single
            # transposes; after the 4th, a DVE copy publishes the xt tile.
            tp_state = {}  # (r, j) -> [psum_tile, count]

            def emit_one_transpose(r, j, i):
                src = xnat[r]
                base = P * 4 * j
                if (r, j) not in tp_state:
                    tp_state[(r, j)] = [tp_p.tile([P, 4 * P], BF16, tag="tp", name="tpps"), 0]
                ps, _ = tp_state[(r, j)]
                nc.tensor.transpose(
                    ps[:, P * i:P * (i + 1)],
                    src[:, base + P * i:base + P * (i + 1)],
                    ident[:],
                )
                tp_state[(r, j)][1] += 1
                if tp_state[(r, j)][1] == 4:
                    xt = xt_p.tile([P, 4 * P], BF16, tag=f"xt{j}", name="xtt")
                    nc.vector.tensor_copy(xt[:], ps[:])
                    xts_all.setdefault(r, {})[j] = xt
                    del tp_state[(r, j)]

            tp_queue = [(r, j, i)
                        for r in range(R_TILES)
                        for j in range(8 if r < 4 else 4)
                        for i in range(4)]
            # pos_end[(r, j)] = queue index once group (r, j) is complete
            pos_end = {}
            for idx, (r_, j_, i_) in enumerate(tp_queue):
                pos_end[(r_, j_)] = idx + 1
            row_end_pos = {r_: 0 for r_ in range(R_TILES)}
            for (r_, j_), e in pos_end.items():
                row_end_pos[r_] = max(row_end_pos[r_], e)
            state = {"cursor": 0}

            def pump_to(idx):
                # emit single transposes up to global index idx (exclusive)
                idx = min(idx, len(tp_queue))
                while state["cursor"] < idx:
                    r_, j_, i_ = tp_queue[state["cursor"]]
                    emit_one_transpose(r_, j_, i_)
                    state["cursor"] += 1

            # j-group needed to cover all d-tiles of block b
            J_HI = [(int(OFFS[b + 1]) - 1) // 512 for b in range(len(SIZES))]

            # process the 2 MiB w1 block LAST in each row-tile: its preload
            # gets ~7 extra us inside the DMA ramp-up window
            B_ORDER = [0, 2, 3, 4, 5, 6, 7, 1]


            # Sync-ring prologue order, matched to PE consumption:
            # x0 first (transposes start ASAP), then the odd chunks of the
            # first two blocks' weights (needed by the earliest matmuls),
            # then x1, then the remaining odd weight chunks, interleaved
            # ahead of the steady-state x prefetches.
            issue_x_dma(0, chunks=4)
            for t, src in w_sync_dmas[0]:
                nc.sync.dma_start(out=t[:], in_=src)
            issue_x_dma(1)
            for i in (2, 3, 4, 5, 6, 7, 1):
                for t, src in w_sync_dmas[i]:
                    nc.sync.dma_start(out=t[:], in_=src)

            issue_xbar_pair(2)

            # budget-driven interleave: one transpose LD (~99 ns) hides
            # under ~240 matmul-stream cycles
            T_COVER_CYCLES = 240
            budget = {"c": 0}

            def interleave_pump(nw, cap):
                budget["c"] += nw
                while (budget["c"] >= T_COVER_CYCLES
                       and state["cursor"] < min(cap, len(tp_queue))):
                    r_, j_, i_ = tp_queue[state["cursor"]]
                    emit_one_transpose(r_, j_, i_)
                    state["cursor"] += 1
                    budget["c"] -= T_COVER_CYCLES

            for r in range(R_TILES):
                last = r == R_TILES - 1
                if r + 2 < R_TILES:
                    issue_x_dma(r + 2)
                if r == 0:
                    issue_xbar_pair(3)
                cap = row_end_pos[min(r + 2, R_TILES - 1)]
                o_t = osb_p.tile([P, D], BF16, tag="os")
                for b in B_ORDER:
                    s = SIZES[b]
                    jn = J_HI[b] if r < 4 else min(J_HI[b], 3)
                    pump_to(pos_end[(r, jn)])
                    xts = xts_all[r]
                    d0 = int(OFFS[b]) // P
                    kt = s // P
                    n0 = 0
                    for nw in E_SLICES[s]:
                        ps = mm_p.tile([P, nw], F32, tag="mm", name="mmps")
                        for k in range(kt):
                            g = d0 + k
                            if r < 4 or g < 16:
                                lhsT = xts[g // 4][:, P * (g % 4):P * (g % 4 + 1)]
                            else:
                                lhsT = xbar_xts[r // 2][g][
                                    :, (r % 2) * P:(r % 2) * P + P]
                            nc.tensor.matmul(
                                ps[:], lhsT, wt[b][k][:, n0:n0 + nw],
                                start=(k == 0), stop=(k == kt - 1),
                            )
                            interleave_pump(nw, cap)
                        dst = o_t[:, int(OFFS[b]) + n0:int(OFFS[b]) + n0 + nw]
                        nc.vector.tensor_copy(dst, ps[:])
                        if last:
                            # stream the final row-tile out per-slice so the
                            # tail store overlaps the remaining compute
                            c0 = int(OFFS[b]) + n0
                            nc.sync.dma_start(out=out_v[r][:, c0:c0 + nw],
                                               in_=o_t[:, c0:c0 + nw])
                        n0 += nw
                del xts_all[r]
                if not last:
                    nc.sync.dma_start(out=out_v[r][:, :D // 2], in_=o_t[:, :D // 2])
                    nc.sync.dma_start(out=out_v[r][:, D // 2:], in_=o_t[:, D // 2:])

    nc.finalize()
    _cache["nc"] = nc
    return nc


def build_in_maps(x, w0, w1, w2, w3, w4, w5, w6, w7):
    x = np.asarray(x, dtype=np.float32).reshape(ROWS_TOTAL, D)
    xb = x.astype(ml_dtypes.bfloat16)
    ws = [w0, w1, w2, w3, w4, w5, w6, w7]
    wts = [
        np.ascontiguousarray(np.asarray(w, dtype=np.float32).T).astype(
            ml_dtypes.bfloat16
        )
        for w in ws
    ]
    ident = np.eye(P, dtype=np.float32).astype(ml_dtypes.bfloat16)
    in_maps = []
    for c in range(N_CORES):
        m = {"x": xb[c * ROWS_PER_CORE:(c + 1) * ROWS_PER_CORE], "ident": ident}
        for i, wtb in enumerate(wts):
            m[f"w{i}"] = wtb
        in_maps.append(m)
    return in_maps


def kernel(x, w0, w1, w2, w3, w4, w5, w6, w7):
    nc = build_nc()
    in_maps = build_in_maps(x, w0, w1, w2, w3, w4, w5, w6, w7)
    res = run_bass_kernel_spmd(nc, in_maps, list(range(N_CORES)))
    out = np.concatenate([r["out"] for r in res.results], axis=0)
    return out.reshape(4, 2048, D).astype(np.float32)



# revision 31
# speedup vs baseline: 1.9535x; 1.6398x over previous
"""Block-diagonal rotation (COB) kernel for Trainium2, 8 NeuronCores.

Computes out[..., block_i] = x[..., block_i] @ W_i.T for 8 square blocks of
sizes [512, 1024, 256, 768, 384, 640, 128, 384] (features sum to 4096),
x shape (4, 2048, 4096) fp32.

Strategy (bf16 end-to-end, data-parallel over rows):
  - 8192 rows split 8 ways (1024 rows/core); each core holds all weights.
  - Host converts x and the pre-transposed weights to bf16 and upcasts the
    bf16 output back to fp32 (harness tolerance is 2e-2; bf16 end-to-end
    lands ~3.9e-3).  This halves HBM traffic vs fp32: 21.1 MiB/core
    (x-in 8 + w 5.1 + out 8) and makes bf16 PE transposes 1 cycle/row.
  - x tiles [128, 4096] are DMA'd naturally (rows on partitions),
    transposed 128x128 on the TensorEngine against a DMA'd identity,
    PSUM->SBUF copied by the DVE, then used as the stationary operand of
    bf16 matmuls against SBUF-resident weight tiles (PSUM fp32 accum).
  - PSUM results are downcast-copied to bf16 staging tiles (alternating
    DVE/ACT) and stored as 0.5 MiB DMAs; the final row-tile streams out
    per-slice so the tail overlaps compute.
  - Transposes are emitted ONE at a time between matmuls (budget pump,
    ~240 matmul-cycles of cover per transpose) with a 2-row-tile
    lookahead window, so their stationary loads hide under matmul
    streaming.
  - Prologue is DMA-ramp limited (~120 GB/s for the first ~25 us), so:
    weights split across both HWDGE rings (even k-chunks on scalar, odd
    on sync interleaved with the x prefetches, in consumption order) and
    the 2 MiB w1 block is processed LAST in each row-tile (B_ORDER),
    giving its preload ~7 us of extra slack inside the ramp window.

Measured on trn2 (8 cores): ~116-120 us HW exec cold (device shows
+-10-15% thermal/neighbor variance; identical builds measured 115.3-134),
max rel err ~3.9e-3.  PE busy ~92-95 us vs an 83.6 us streaming floor
(167,936 matmul rows + 32,768 transpose rows @ 2.4 GHz); remaining loss
is the fixed ~9 us framework init, DMA-ramp-limited prologue stalls
(~5 us), and the ~6 us teardown drain.
"""

import numpy as np
import ml_dtypes

import concourse.bacc as bacc
import concourse.mybir as mybir
from concourse.tile import TileContext
from concourse.bass_utils import run_bass_kernel_spmd

SIZES = [512, 1024, 256, 768, 384, 640, 128, 384]
OFFS = np.cumsum([0] + SIZES)
N_CORES = 8
ROWS_TOTAL = 4 * 2048
ROWS_PER_CORE = ROWS_TOTAL // N_CORES  # 1024
D = 4096
P = 128
R_TILES = ROWS_PER_CORE // P  # 8

# e-slices per block: PSUM bank holds 512 fp32 per partition
E_SLICES = {
    512: [512], 1024: [512, 512], 256: [256], 768: [512, 256],
    384: [384], 640: [384, 256], 128: [128],
}

BF16 = mybir.dt.bfloat16
F32 = mybir.dt.float32

_cache = {}


def build_nc():
    if "nc" in _cache:
        return _cache["nc"]
    nc = bacc.Bacc()
    x_d = nc.declare_dram_parameter("x", [ROWS_PER_CORE, D], BF16, isOutput=False)
    w_d = [
        nc.declare_dram_parameter(f"w{i}", [s, s], BF16, isOutput=False)
        for i, s in enumerate(SIZES)
    ]
    id_d = nc.declare_dram_parameter("ident", [P, P], BF16, isOutput=False)
    out_d = nc.declare_dram_parameter("out", [ROWS_PER_CORE, D], BF16, isOutput=True)

    x_v = x_d.rearrange("(r p) d -> r p d", p=P)
    out_v = out_d.rearrange("(r p) d -> r p d", p=P)

    with TileContext(nc) as tc:
        with (
            tc.tile_pool(name="wres", bufs=1) as wres,
            tc.tile_pool(name="xnat", bufs=2) as xnat_p,
            tc.tile_pool(name="xt", bufs=3) as xt_p,
            tc.tile_pool(name="osb", bufs=2) as osb_p,
            tc.tile_pool(name="idp", bufs=1) as idp,
            tc.tile_pool(name="tp", bufs=2, space="PSUM") as tp_p,
            tc.tile_pool(name="mm", bufs=4, space="PSUM") as mm_p,
        ):
            # identity (bf16) for PE transpose — DMA'd from DRAM so the
            # first transpose doesn't wait on DVE table loads / iota setup
            ident = idp.tile([P, P], BF16, tag="idb")
            nc.scalar.dma_start(out=ident[:], in_=id_d[:, :])

            # resident weights: per block, per k-tile: [128, s] bf16.
            # Even k-chunks stream on the scalar ring immediately (it is
            # otherwise idle in the prologue); odd k-chunks go on the sync
            # ring interleaved between the x-tile loads (deferred below) so
            # each half-weight stream finishes just ahead of first use.
            wt = [None] * len(SIZES)
            w_sync_dmas = {i: [] for i in range(len(SIZES))}
            ci = 0
            for i in [0, 2, 3, 4, 5, 6, 7, 1]:
                s = SIZES[i]
                w_v = w_d[i].rearrange("(k p) e -> k p e", p=P)
                ks = []
                for k in range(s // P):
                    t = wres.tile([P, s], BF16, tag=f"w{i}_{k}")
                    if ci % 2 == 0:
                        nc.scalar.dma_start(out=t[:], in_=w_v[k])
                    else:
                        w_sync_dmas[i].append((t, w_v[k]))
                    ks.append(t)
                    ci += 1
                wt[i] = ks

            # Software pipeline over row-tiles (demand-driven transpose pump,
            # sliding window of up to 2 row-tiles of transposed x).
            xnat = {}
            xts_all = {}

            def issue_x_dma(r, chunks=2):
                xn = xnat_p.tile([P, D], BF16, tag="xn", name="xnt")
                q = D // chunks
                for c in range(chunks):
                    nc.sync.dma_start(out=xn[:, c * q:(c + 1) * q],
                                      in_=x_v[r][:, c * q:(c + 1) * q])
                xnat[r] = xn

            # Transposes are LDWEIGHTS-bound on the PE (the 128-row
            # stationary load takes ~2x the 128-row identity stream), so we
            # emit them ONE at a time interleaved between block matmuls:
            # each matmul with nw>=256 fully hides one transpose's LD.
            # State: per (r, j) group, a psum tile filled by 4 single
            # transposes; after the 4th, a DVE copy publishes the xt tile.
            tp_state = {}  # (r, j) -> [psum_tile, count]

            def emit_one_transpose(r, j, i):
                src = xnat[r]
                base = P * 4 * j
                if (r, j) not in tp_state:
                    tp_state[(r, j)] = [tp_p.tile([P, 4 * P], BF16, tag="tp", name="tpps"), 0]
                ps, _ = tp_state[(r, j)]
                nc.tensor.transpose(
                    ps[:, P * i:P * (i + 1)],
                    src[:, base + P * i:base + P * (i + 1)],
                    ident[:],
                )
                tp_state[(r, j)][1] += 1
                if tp_state[(r, j)][1] == 4:
                    xt = xt_p.tile([P, 4 * P], BF16, tag=f"xt{j}", name="xtt")
                    nc.vector.tensor_copy(xt[:], ps[:])
                    xts_all.setdefault(r, {})[j] = xt
                    del tp_state[(r, j)]

            tp_queue = [(r, j, i)
                        for r in range(R_TILES) for j in range(8) for i in range(4)]
            state = {"cursor": 0}

            def pump_to(idx):
                # emit single transposes up to global index idx (exclusive)
                idx = min(idx, len(tp_queue))
                while state["cursor"] < idx:
                    r_, j_, i_ = tp_queue[state["cursor"]]
                    emit_one_transpose(r_, j_, i_)
                    state["cursor"] += 1

            # j-group needed to cover all d-tiles of block b
            J_HI = [(int(OFFS[b + 1]) - 1) // 512 for b in range(len(SIZES))]

            # process the 2 MiB w1 block LAST in each row-tile: its preload
            # gets ~7 extra us inside the DMA ramp-up window
            B_ORDER = [0, 2, 3, 4, 5, 6, 7, 1]


            # Sync-ring prologue order, matched to PE consumption:
            # x0 first (transposes start ASAP), then the odd chunks of the
            # first two blocks' weights (needed by the earliest matmuls),
            # then x1, then the remaining odd weight chunks, interleaved
            # ahead of the steady-state x prefetches.
            issue_x_dma(0, chunks=4)
            for t, src in w_sync_dmas[0]:
                nc.sync.dma_start(out=t[:], in_=src)
            issue_x_dma(1)
            for i in (2, 3, 4, 5, 6, 7, 1):
                for t, src in w_sync_dmas[i]:
                    nc.sync.dma_start(out=t[:], in_=src)

            # budget-driven interleave: one transpose LD (~99 ns) hides
            # under ~240 matmul-stream cycles
            T_COVER_CYCLES = 240
            budget = {"c": 0}

            def interleave_pump(nw, cap):
                budget["c"] += nw
                while (budget["c"] >= T_COVER_CYCLES
                       and state["cursor"] < min(cap, len(tp_queue))):
                    r_, j_, i_ = tp_queue[state["cursor"]]
                    emit_one_transpose(r_, j_, i_)
                    state["cursor"] += 1
                    budget["c"] -= T_COVER_CYCLES

            for r in range(R_TILES):
                last = r == R_TILES - 1
                if r + 2 < R_TILES:
                    issue_x_dma(r + 2)
                cap = (r + 3) * 32  # transposes only for row-tiles <= r+2
                o_t = osb_p.tile([P, D], BF16, tag="os")
                for b in B_ORDER:
                    s = SIZES[b]
                    pump_to(4 * (r * 8 + J_HI[b] + 1))
                    xts = xts_all[r]
                    d0 = int(OFFS[b]) // P
                    kt = s // P
                    n0 = 0
                    for nw in E_SLICES[s]:
                        ps = mm_p.tile([P, nw], F32, tag="mm", name="mmps")
                        for k in range(kt):
                            g = d0 + k
                            lhsT = xts[g // 4][:, P * (g % 4):P * (g % 4 + 1)]
                            nc.tensor.matmul(
                                ps[:], lhsT, wt[b][k][:, n0:n0 + nw],
                                start=(k == 0), stop=(k == kt - 1),
                            )
                            interleave_pump(nw, cap)
                        dst = o_t[:, int(OFFS[b]) + n0:int(OFFS[b]) + n0 + nw]
                        if (r + b) % 2 == 0:
                            nc.scalar.copy(dst, ps[:])
                        else:
                            nc.vector.tensor_copy(dst, ps[:])
                        if last:
                            # stream the final row-tile out per-slice so the
                            # tail store overlaps the remaining compute
                            c0 = int(OFFS[b]) + n0
                            nc.sync.dma_start(out=out_v[r][:, c0:c0 + nw],
                                               in_=o_t[:, c0:c0 + nw])
                        n0 += nw
                del xts_all[r]
                if not last:
                    nc.sync.dma_start(out=out_v[r][:, :D // 2], in_=o_t[:, :D // 2])
                    nc.sync.dma_start(out=out_v[r][:, D // 2:], in_=o_t[:, D // 2:])

    nc.finalize()
    _cache["nc"] = nc
    return nc


def build_in_maps(x, w0, w1, w2, w3, w4, w5, w6, w7):
    x = np.asarray(x, dtype=np.float32).reshape(ROWS_TOTAL, D)
    xb = x.astype(ml_dtypes.bfloat16)
    ws = [w0, w1, w2, w3, w4, w5, w6, w7]
    wts = [
        np.ascontiguousarray(np.asarray(w, dtype=np.float32).T).astype(
            ml_dtypes.bfloat16
        )
        for w in ws
    ]
    ident = np.eye(P, dtype=np.float32).astype(ml_dtypes.bfloat16)
    in_maps = []
    for c in range(N_CORES):
        m = {"x": xb[c * ROWS_PER_CORE:(c + 1) * ROWS_PER_CORE], "ident": ident}
        for i, wtb in enumerate(wts):
            m[f"w{i}"] = wtb
        in_maps.append(m)
    return in_maps


def kernel(x, w0, w1, w2, w3, w4, w5, w6, w7):
    nc = build_nc()
    in_maps = build_in_maps(x, w0, w1, w2, w3, w4, w5, w6, w7)
    res = run_bass_kernel_spmd(nc, in_maps, list(range(N_CORES)))
    out = np.concatenate([r["out"] for r in res.results], axis=0)
    return out.reshape(4, 2048, D).astype(np.float32)



# revision 32
# speedup vs baseline: 1.9696x; 1.0083x over previous
"""Block-diagonal rotation (COB) kernel for Trainium2, 8 NeuronCores.

Computes out[..., block_i] = x[..., block_i] @ W_i.T for 8 square blocks of
sizes [512, 1024, 256, 768, 384, 640, 128, 384] (features sum to 4096),
x shape (4, 2048, 4096) fp32.

Strategy (bf16 end-to-end, data-parallel over rows):
  - 8192 rows split 8 ways (1024 rows/core); each core holds all weights.
  - Host converts x and the pre-transposed weights to bf16 and upcasts the
    bf16 output back to fp32 (harness tolerance is 2e-2; bf16 end-to-end
    lands ~3.9e-3).  This halves HBM traffic vs fp32: 21.1 MiB/core
    (x-in 8 + w 5.1 + out 8) and makes bf16 PE transposes 1 cycle/row.
  - x tiles [128, 4096] are DMA'd naturally (rows on partitions),
    transposed 128x128 on the TensorEngine against a DMA'd identity,
    PSUM->SBUF copied by the DVE, then used as the stationary operand of
    bf16 matmuls against SBUF-resident weight tiles (PSUM fp32 accum).
  - PSUM results are downcast-copied to bf16 staging tiles (alternating
    DVE/ACT) and stored as 0.5 MiB DMAs; the final row-tile streams out
    per-slice so the tail overlaps compute.
  - Transposes are emitted ONE at a time between matmuls (budget pump,
    ~240 matmul-cycles of cover per transpose) with a 2-row-tile
    lookahead window, so their stationary loads hide under matmul
    streaming.
  - Prologue is DMA-ramp limited (~120 GB/s for the first ~25 us), so:
    weights split across both HWDGE rings (even k-chunks on scalar, odd
    on sync interleaved with the x prefetches, in consumption order) and
    the 2 MiB w1 block is processed LAST in each row-tile (B_ORDER),
    giving its preload ~7 us of extra slack inside the ramp window.

Measured on trn2 (8 cores): ~116-120 us HW exec cold (device shows
+-10-15% thermal/neighbor variance; identical builds measured 115.3-134),
max rel err ~3.9e-3.  PE busy ~92-95 us vs an 83.6 us streaming floor
(167,936 matmul rows + 32,768 transpose rows @ 2.4 GHz); remaining loss
is the fixed ~9 us framework init, DMA-ramp-limited prologue stalls
(~5 us), and the ~6 us teardown drain.
"""

import numpy as np
import ml_dtypes

import concourse.bacc as bacc
import concourse.mybir as mybir
from concourse.tile import TileContext
from concourse.bass_utils import run_bass_kernel_spmd

SIZES = [512, 1024, 256, 768, 384, 640, 128, 384]
OFFS = np.cumsum([0] + SIZES)
N_CORES = 8
ROWS_TOTAL = 4 * 2048
ROWS_PER_CORE = ROWS_TOTAL // N_CORES  # 1024
D = 4096
P = 128
R_TILES = ROWS_PER_CORE // P  # 8

# e-slices per block: PSUM bank holds 512 fp32 per partition
E_SLICES = {
    512: [512], 1024: [512, 512], 256: [256], 768: [512, 256],
    384: [384], 640: [384, 256], 128: [128],
}

BF16 = mybir.dt.bfloat16
F32 = mybir.dt.float32

_cache = {}


def build_nc():
    if "nc" in _cache:
        return _cache["nc"]
    nc = bacc.Bacc()
    x_d = nc.declare_dram_parameter("x", [ROWS_PER_CORE, D], BF16, isOutput=False)
    w_d = [
        nc.declare_dram_parameter(f"w{i}", [s, s], BF16, isOutput=False)
        for i, s in enumerate(SIZES)
    ]
    id_d = nc.declare_dram_parameter("ident", [P, P], BF16, isOutput=False)
    out_d = nc.declare_dram_parameter("out", [ROWS_PER_CORE, D], BF16, isOutput=True)

    x_v = x_d.rearrange("(r p) d -> r p d", p=P)
    out_v = out_d.rearrange("(r p) d -> r p d", p=P)

    with TileContext(nc) as tc:
        with (
            tc.tile_pool(name="wres", bufs=1) as wres,
            tc.tile_pool(name="xnat", bufs=2) as xnat_p,
            tc.tile_pool(name="xt", bufs=3) as xt_p,
            tc.tile_pool(name="osb", bufs=2) as osb_p,
            tc.tile_pool(name="idp", bufs=1) as idp,
            tc.tile_pool(name="tp", bufs=2, space="PSUM") as tp_p,
            tc.tile_pool(name="mm", bufs=4, space="PSUM") as mm_p,
        ):
            # identity (bf16) for PE transpose — DMA'd from DRAM so the
            # first transpose doesn't wait on DVE table loads / iota setup
            ident = idp.tile([P, P], BF16, tag="idb")
            nc.scalar.dma_start(out=ident[:], in_=id_d[:, :])

            # resident weights: per block, per k-tile: [128, s] bf16.
            # Even k-chunks stream on the scalar ring immediately (it is
            # otherwise idle in the prologue); odd k-chunks go on the sync
            # ring interleaved between the x-tile loads (deferred below) so
            # each half-weight stream finishes just ahead of first use.
            wt = [None] * len(SIZES)
            w_sync_dmas = {i: [] for i in range(len(SIZES))}
            ci = 0
            for i in [0, 2, 3, 4, 5, 6, 7, 1]:
                s = SIZES[i]
                w_v = w_d[i].rearrange("(k p) e -> k p e", p=P)
                ks = []
                for k in range(s // P):
                    t = wres.tile([P, s], BF16, tag=f"w{i}_{k}")
                    if ci % 2 == 0:
                        nc.scalar.dma_start(out=t[:], in_=w_v[k])
                    else:
                        w_sync_dmas[i].append((t, w_v[k]))
                    ks.append(t)
                    ci += 1
                wt[i] = ks

            # Software pipeline over row-tiles (demand-driven transpose pump,
            # sliding window of up to 2 row-tiles of transposed x).
            xnat = {}
            xts_all = {}

            def issue_x_dma(r, chunks=2):
                xn = xnat_p.tile([P, D], BF16, tag="xn", name="xnt")
                q = D // chunks
                for c in range(chunks):
                    nc.sync.dma_start(out=xn[:, c * q:(c + 1) * q],
                                      in_=x_v[r][:, c * q:(c + 1) * q])
                xnat[r] = xn

            # Transposes are LDWEIGHTS-bound on the PE (the 128-row
            # stationary load takes ~2x the 128-row identity stream), so we
            # emit them ONE at a time interleaved between block matmuls:
            # each matmul with nw>=256 fully hides one transpose's LD.
            # State: per (r, j) group, a psum tile filled by 4 single
            # transposes; after the 4th, a DVE copy publishes the xt tile.
            tp_state = {}  # (r, j) -> [psum_tile, count]

            def emit_one_transpose(r, j, i):
                src = xnat[r]
                base = P * 4 * j
                if (r, j) not in tp_state:
                    tp_state[(r, j)] = [tp_p.tile([P, 4 * P], BF16, tag="tp", name="tpps"), 0]
                ps, _ = tp_state[(r, j)]
                nc.tensor.transpose(
                    ps[:, P * i:P * (i + 1)],
                    src[:, base + P * i:base + P * (i + 1)],
                    ident[:],
                )
                tp_state[(r, j)][1] += 1
                if tp_state[(r, j)][1] == 4:
                    xt = xt_p.tile([P, 4 * P], BF16, tag=f"xt{j}", name="xtt")
                    nc.vector.tensor_copy(xt[:], ps[:])
                    xts_all.setdefault(r, {})[j] = xt
                    del tp_state[(r, j)]

            tp_queue = [(r, j, i)
                        for r in range(R_TILES) for j in range(8) for i in range(4)]
            state = {"cursor": 0}

            def pump_to(idx):
                # emit single transposes up to global index idx (exclusive)
                idx = min(idx, len(tp_queue))
                while state["cursor"] < idx:
                    r_, j_, i_ = tp_queue[state["cursor"]]
                    emit_one_transpose(r_, j_, i_)
                    state["cursor"] += 1

            # j-group needed to cover all d-tiles of block b
            J_HI = [(int(OFFS[b + 1]) - 1) // 512 for b in range(len(SIZES))]

            # process the 2 MiB w1 block LAST in each row-tile: its preload
            # gets ~7 extra us inside the DMA ramp-up window
            B_ORDER = [0, 2, 3, 4, 5, 6, 7, 1]


            # Sync-ring prologue order, matched to PE consumption:
            # x0 first (transposes start ASAP), then the odd chunks of the
            # first two blocks' weights (needed by the earliest matmuls),
            # then x1, then the remaining odd weight chunks, interleaved
            # ahead of the steady-state x prefetches.
            issue_x_dma(0, chunks=4)
            for t, src in w_sync_dmas[0]:
                nc.sync.dma_start(out=t[:], in_=src)
            issue_x_dma(1)
            for i in (2, 3, 4, 5, 6, 7, 1):
                for t, src in w_sync_dmas[i]:
                    nc.sync.dma_start(out=t[:], in_=src)

            # budget-driven interleave: one transpose LD (~99 ns) hides
            # under ~240 matmul-stream cycles
            T_COVER_CYCLES = 240
            budget = {"c": 0}

            def interleave_pump(nw, cap):
                budget["c"] += nw
                while (budget["c"] >= T_COVER_CYCLES
                       and state["cursor"] < min(cap, len(tp_queue))):
                    r_, j_, i_ = tp_queue[state["cursor"]]
                    emit_one_transpose(r_, j_, i_)
                    state["cursor"] += 1
                    budget["c"] -= T_COVER_CYCLES

            for r in range(R_TILES):
                last = r == R_TILES - 1
                if r + 2 < R_TILES:
                    issue_x_dma(r + 2)
                cap = (r + 3) * 32  # transposes only for row-tiles <= r+2
                o_t = osb_p.tile([P, D], BF16, tag="os")
                for b in B_ORDER:
                    s = SIZES[b]
                    pump_to(4 * (r * 8 + J_HI[b] + 1))
                    xts = xts_all[r]
                    d0 = int(OFFS[b]) // P
                    kt = s // P
                    n0 = 0
                    for nw in E_SLICES[s]:
                        ps = mm_p.tile([P, nw], F32, tag="mm", name="mmps")
                        for k in range(kt):
                            g = d0 + k
                            lhsT = xts[g // 4][:, P * (g % 4):P * (g % 4 + 1)]
                            nc.tensor.matmul(
                                ps[:], lhsT, wt[b][k][:, n0:n0 + nw],
                                start=(k == 0), stop=(k == kt - 1),
                            )
                            interleave_pump(nw, cap)
                        dst = o_t[:, int(OFFS[b]) + n0:int(OFFS[b]) + n0 + nw]
                        if (r + b) % 2 == 0:
                            nc.scalar.copy(dst, ps[:])
                        else:
                            nc.vector.tensor_copy(dst, ps[:])
                        if last:
                            # stream the final row-tile out per-slice so the
                            # tail store overlaps the remaining compute
                            c0 = int(OFFS[b]) + n0
                            nc.sync.dma_start(out=out_v[r][:, c0:c0 + nw],
                                               in_=o_t[:, c0:c0 + nw])
                        n0 += nw
                    if r < 2:
                        # ramp phase: weight DMAs are the binding constraint;
                        # front-load transposes for row-tiles 0-1 (their x is
                        # already resident) so weight-wait gaps become useful
                        # PE work and the p-state stays hot
                        pump_to(min(state["cursor"] + 8, 64))
                del xts_all[r]
                if not last:
                    nc.sync.dma_start(out=out_v[r][:, :D // 2], in_=o_t[:, :D // 2])
                    nc.sync.dma_start(out=out_v[r][:, D // 2:], in_=o_t[:, D // 2:])

    nc.finalize()
    _cache["nc"] = nc
    return nc


def build_in_maps(x, w0, w1, w2, w3, w4, w5, w6, w7):
    x = np.asarray(x, dtype=np.float32).reshape(ROWS_TOTAL, D)
    xb = x.astype(ml_dtypes.bfloat16)
    ws = [w0, w1, w2, w3, w4, w5, w6, w7]
    wts = [
        np.ascontiguousarray(np.asarray(w, dtype=np.float32).T).astype(
            ml_dtypes.bfloat16
        )
        for w in ws
    ]
    ident = np.eye(P, dtype=np.float32).astype(ml_dtypes.bfloat16)
    in_maps = []
    for c in range(N_CORES):
        m = {"x": xb[c * ROWS_PER_CORE:(c + 1) * ROWS_PER_CORE], "ident": ident}
        for i, wtb in enumerate(wts):
            m[f"w{i}"] = wtb
        in_maps.append(m)
    return in_maps


def kernel(x, w0, w1, w2, w3, w4, w5, w6, w7):
    nc = build_nc()
    in_maps = build_in_maps(x, w0, w1, w2, w3, w4, w5, w6, w7)
    res = run_bass_kernel_spmd(nc, in_maps, list(range(N_CORES)))
    out = np.concatenate([r["out"] for r in res.results], axis=0)
    return out.reshape(4, 2048, D).astype(np.float32)

